# revision 38
# baseline (speedup 1.0000x reference)
"""Causal single-head attention (B=256, T=256, C=384, H=64) on 8 trn2 cores.

Data-parallel over batch: each core computes 32 batches independently.

Design (all-bf16 storage, fp32 PSUM accumulate):
  - Host stages x as bf16 in a DMA-friendly layout [8, 128, 4, 3, 256]
    (8 macro-groups of 4 batches; each partition row is 6KB contiguous),
    so input loads are 8 big DMAs instead of 96 small ones. The first
    macro is loaded per-batch so the pipeline fills early.
  - Q and K are projected in ONE packed matmul per (batch, c-chunk):
    stationary Wqk [128c, (q64|k64)] -> PSUM [q rows 0:64 | k rows 64:128].
    One full-width DVE copy drains PSUM to SBUF bf16; a cheap all-SBUF
    partition-shifted copy then brings the k half down to base 0.
  - V is projected in natural [t, h] layout (stationary xT chunk, moving Wv)
    -> no V transpose. The SBUF copy targets v_aug[..., 0:64] whose ones
    column (v_aug[..., 64]=1, prefilled outside the loop) makes the AV
    matmul's last output column the softmax denominator.
  - Scores are computed TRANSPOSED: weiT[s,t] = k q^T (stationary kT chunk,
    moving qT), so exp(weiT) written by ACT directly yields pT in SBUF --
    no P transposes. Causal masking is done multiplicatively AFTER exp
    (logits are bounded so exp never overflows), split between DVE (s0
    diag, gates AV's t0 matmul) and Pool (s1 diag, latency hidden); it
    also fixes the rowsums (masked entries contribute 0).
  - Causal block structure: s0 chunk is needed by all t (free dim 256);
    s1 chunk only by t1 (free 128). The (s1, t0) quadrant of weiT2 is never
    computed or exp'd -- its PSUM space is reused as the AV output tile
    (o lives at weiT2[:, b2, 1, 0:130]), saving a PSUM bank and giving the
    AV output double-buffering for free.
  - The AV output leaves UNNORMALIZED together with its rowsum column
    (65 cols); the softmax division happens on the host after the gather.
    Output accumulates in SBUF bf16 and leaves in overlapped Pool-SWDGE
    DMAs; host transposes back, divides, casts to f32.
  - The main loop is software-pipelined with a one-macro skew: PE order
    per macro is weiT_h0, weiT_h1, proj_qk', AV_h0, proj_qk'', proj_v',
    AV_h1 so exp/mask latency hides under next-macro projection work.
    PSUM accumulation pairs are kept strictly consecutive (interleaving a
    complete start/stop group inside a pending pair corrupts it on HW).

Engine budget per 4-batch macro (cost model): PE ~2885ns, DVE ~2880ns,
ACT ~2590ns, Pool/DMA lower -- PE-bound at ~23us busy, ~41us critical
path (fill + drain + sem latency) per core program.
"""

import os
import sys

import numpy as np

for _p in ("/opt/trn_rl_repo",):
    if _p not in sys.path:
        sys.path.insert(0, _p)

B, T, C, H = 256, 256, 384, 64
N_CORES = 8
BPC = B // N_CORES  # batches per core (32)
P = 128
M4 = BPC // 4  # 4-batch macro groups per core (8)

LAST_RESULT = None  # BassKernelResults of the most recent run (for test.py)


def _build_nc(bpc=BPC, repeats=1):
    import concourse.bacc as bacc
    import concourse.mybir as mybir
    import concourse.tile as tile

    f32 = mybir.dt.float32
    bf16 = mybir.dt.bfloat16
    m4 = bpc // 4

    nc = bacc.Bacc("TRN2", target_bir_lowering=False, debug=False)

    xt = nc.dram_tensor("xt", [m4, P, 4, 3, T], bf16, kind="ExternalInput")
    wqk = nc.dram_tensor("wqk", [P, 3, P], bf16, kind="ExternalInput")
    wv = nc.dram_tensor("wv", [P, 3, H], bf16, kind="ExternalInput")
    out = nc.dram_tensor("out", [P, bpc, 2, H + 1], bf16, kind="ExternalOutput")

    Exp = mybir.ActivationFunctionType.Exp
    mult_op = mybir.AluOpType.mult

    with tile.TileContext(nc) as tc:
        with (
            tc.tile_pool(name="consts", bufs=1) as consts,
            tc.tile_pool(name="inp", bufs=4) as inp,
            tc.tile_pool(name="sb", bufs=3) as sb,
            tc.tile_pool(name="pp", bufs=4) as pp,
            tc.tile_pool(name="va_pool", bufs=2) as va_pool,
            tc.tile_pool(name="acc", bufs=1) as acc,
            tc.tile_pool(name="ps_qk", bufs=1, space="PSUM") as ps_qk,
            tc.tile_pool(name="ps_v", bufs=2, space="PSUM") as ps_v,
            tc.tile_pool(name="ps_w", bufs=2, space="PSUM") as ps_w,
        ):
            # 0/1 upper-triangular (incl diag) bf16 mask: keep s <= t.
            mask01 = consts.tile([P, P], bf16)
            nc.gpsimd.memset(mask01, 1.0)
            nc.gpsimd.affine_select(
                out=mask01,
                in_=mask01,
                compare_op=mybir.AluOpType.is_ge,
                fill=0.0,
                base=0,
                pattern=[[1, P]],
                channel_multiplier=-1,
            )

            wqk_sb = consts.tile([P, 3, P], bf16)
            nc.sync.dma_start(wqk_sb, wqk[:])
            wv_sb = consts.tile([P, 3, H], bf16)

            # Trigger the ACT exp-table load during pipeline fill instead of
            # stalling the first real exp.
            warm = consts.tile([P, 1], f32)
            nc.scalar.activation(warm, mask01[:, 0:1], Exp)

            out_acc = acc.tile([P, bpc, 2, H + 1], bf16)

            # Prefill the ones column of both v_aug buffers; in-loop copies
            # only touch [..., 0:64], so the column survives rotation.
            for _ in range(2):
                va_init = va_pool.tile([P, 4, 2, H + 1], bf16, tag="va")
                nc.gpsimd.memset(va_init[:, :, :, H:H + 1], 1.0)

            import contextlib

            rep_ctx = (
                tc.For_i(0, repeats, 1, hint_engines=(mybir.EngineType.PE,
                                                      mybir.EngineType.DVE,
                                                      mybir.EngineType.Activation,
                                                      mybir.EngineType.Pool,
                                                      mybir.EngineType.SP))
                if repeats > 1
                else contextlib.nullcontext()
            )
            def emit_x_dma(m, split_dma=False):
                x4 = inp.tile([P, 4, 3, T], bf16, tag="x4", name=f"x4_{m}")
                if split_dma:
                    for b in range(4):
                        nc.sync.dma_start(x4[:, b], xt[m, :, b])
                else:
                    nc.sync.dma_start(x4, xt[m])
                return x4

            def alloc_qk4(m):
                return ps_qk.tile([P, 4, T], f32, tag="qk", name=f"qk4_{m}")

            def xsel(m, x4, b):
                return x4[:, b]

            def emit_proj_qk(m, x4, qk4, bs):
                for b in bs:
                    xs = xsel(m, x4, b)
                    for cc in range(3):
                        nc.tensor.matmul(
                            qk4[:, b, :], wqk_sb[:, cc, :], xs[:, cc, :],
                            start=(cc == 0), stop=(cc == 2),
                        )

            def alloc_qk_sb(m):
                qk_sb = sb.tile([P, 4, T], bf16, tag="qk_sb", name=f"qks_{m}")
                k_sb = sb.tile([H, 4, T], bf16, tag="k", name=f"k_{m}")
                return qk_sb, k_sb

            def emit_qk_copy_half(qk4, qk_sb, k_sb, h):
                sl = slice(2 * h, 2 * h + 2)
                nc.vector.tensor_copy(qk_sb[:, sl, :], qk4[:, sl, :])
                nc.vector.tensor_copy(k_sb[:, sl, :], qk_sb[H:P, sl, :])

            def emit_proj_v(m, x4, bs=range(4), v4=None):
                if v4 is None:
                    v4 = ps_v.tile([P, 4, 2, H], f32, tag="v4",
                                   name=f"v4_{m}")
                for b in bs:
                    xs = xsel(m, x4, b)
                    for j in range(2):
                        for cc in range(3):
                            nc.tensor.matmul(
                                v4[:, b, j, :],
                                xs[:, cc, j * P:(j + 1) * P],
                                wv_sb[:, cc, :],
                                start=(cc == 0), stop=(cc == 2),
                            )
                return v4

            def emit_va(m, v4, bs=slice(0, 4), v_aug=None):
                if v_aug is None:
                    v_aug = va_pool.tile([P, 4, 2, H + 1], bf16, tag="va",
                                         name=f"va_{m}")
                nc.scalar.copy(v_aug[:, bs, :, 0:H], v4[:, bs])
                return v_aug

            def emit_weiT(m, half, qk_sb, k_sb):
                weiT2 = ps_w.tile([P, 2, 2, T], f32, tag="w2",
                                  name=f"w2_{m}_{half}")
                for b2 in range(2):
                    b = 2 * half + b2
                    nc.tensor.matmul(  # s0 chunk: all t (free 256)
                        weiT2[:, b2, 0, :],
                        k_sb[:, b, 0:P], qk_sb[0:H, b, :],
                        start=True, stop=True,
                    )
                    nc.tensor.matmul(  # s1 chunk: t1 only (free 128)
                        weiT2[:, b2, 1, P:T],
                        k_sb[:, b, P:T], qk_sb[0:H, b, P:T],
                        start=True, stop=True,
                    )
                return weiT2

            def emit_exp(m, half, weiT2):
                pT2 = pp.tile([P, 2, 2, T], bf16, tag="p2",
                              name=f"p2_{m}_{half}")
                # s1-diag first: it gates the Pool mask (the longer pole)
                nc.scalar.activation(
                    pT2[:, :, 1, P:T], weiT2[:, :, 1, P:T], Exp
                )
                nc.scalar.activation(pT2[:, :, 0, :], weiT2[:, :, 0, :], Exp)
                return pT2

            def emit_mask(pT2, j, engine):
                blk = pT2[:, :, j, j * P:(j + 1) * P]
                engine.tensor_tensor(
                    blk, blk,
                    mask01[:, None, :].to_broadcast((P, 2, P)),
                    mult_op,
                )

            def emit_AV(m, half, weiT2, pT2, v_aug):
                # o (AV output incl rowsum col) is carved out of the
                # never-computed (s1, t0) quadrant of weiT2.
                o2 = weiT2[:, :, 1, 0:2 * (H + 1)].rearrange(
                    "p b (j c) -> p b j c", j=2
                )  # [128, 2, 2, 65]
                for b2 in range(2):
                    b = 2 * half + b2
                    # PSUM accumulation pairs MUST be consecutive: a complete
                    # start/stop group interleaved inside a pending pair
                    # corrupts it (verified on HW). Order [t1s0, t1s1, t0]:
                    # pair consecutive AND t0 (latest dep, mask j0) last.
                    nc.tensor.matmul(
                        o2[:, b2, 1, :], pT2[:, b2, 0, P:T],
                        v_aug[:, b, 0, :], start=True, stop=False,
                    )
                    nc.tensor.matmul(
                        o2[:, b2, 1, :], pT2[:, b2, 1, P:T],
                        v_aug[:, b, 1, :], start=False, stop=True,
                    )
                    nc.tensor.matmul(
                        o2[:, b2, 0, :], pT2[:, b2, 0, 0:P],
                        v_aug[:, b, 0, :], start=True, stop=True,
                    )
                return o2

            def emit_scale(m, half, o2):
                # Unnormalized AV output + rowsum column; the softmax
                # division happens on the host after the gather.
                bb = 4 * m + 2 * half
                nc.vector.tensor_copy(out_acc[:, bb:bb + 2, :, :], o2)

            with rep_ctx:
                # ---- prologue: fill the pipe with macro 0 ----
                x4 = emit_x_dma(0, split_dma=True)
                nc.sync.dma_start(wv_sb, wv[:])  # needed later than wqk/x0
                qk4 = alloc_qk4(0)
                qk_sb, k_sb = alloc_qk_sb(0)
                # interleave proj with per-batch drains so the first weiT
                # starts as soon as batches 0-1 are through the pipe
                for b in range(4):
                    emit_proj_qk(0, x4, qk4, (b,))
                    nc.vector.tensor_copy(qk_sb[:, b:b + 1, :],
                                          qk4[:, b:b + 1, :])
                    nc.vector.tensor_copy(k_sb[:, b:b + 1, :],
                                          qk_sb[H:P, b:b + 1, :])
                v4 = emit_proj_v(0, x4, bs=(0, 1))
                v_aug = emit_va(0, v4, bs=slice(0, 2))
                x4n_pre = emit_x_dma(1)  # prefetch macro 1 input
                v_rest_pending = True

                # ---- steady state: W/A of m interleaved with P of m+1 ----
                # PE order/macro: weiT_h0, weiT_h1, proj_qk', AV_h0,
                # proj_v', AV_h1 -- dependent pairs separated by filler so
                # exp/mask latency is hidden.  DVE order: mask0_h0, qk_copy',
                # k_shift', mask0_h1, recip/scale x2.
                for m in range(m4):
                    w_h0 = emit_weiT(m, 0, qk_sb, k_sb)
                    p_h0 = emit_exp(m, 0, w_h0)
                    emit_mask(p_h0, 0, nc.vector)
                    emit_mask(p_h0, 1, nc.gpsimd)
                    if m == 0 and v_rest_pending:
                        # macro-0 V second half as filler under exp latency
                        emit_proj_v(0, x4, bs=(2, 3), v4=v4)
                        emit_va(0, v4, bs=slice(2, 4), v_aug=v_aug)
                        v_rest_pending = False
                    w_h1 = emit_weiT(m, 1, qk_sb, k_sb)
                    p_h1 = emit_exp(m, 1, w_h1)
                    if m + 1 < m4:
                        x4n = x4n_pre
                        if m + 2 < m4:
                            x4n_pre = emit_x_dma(m + 2)
                        qk4n = alloc_qk4(m + 1)
                        emit_proj_qk(m + 1, x4n, qk4n, (0, 1))
                        qk_sbn, k_sbn = alloc_qk_sb(m + 1)
                        emit_qk_copy_half(qk4n, qk_sbn, k_sbn, 0)
                    o_h0 = emit_AV(m, 0, w_h0, p_h0, v_aug)
                    if m + 1 < m4:
                        emit_proj_qk(m + 1, x4n, qk4n, (2, 3))
                        emit_qk_copy_half(qk4n, qk_sbn, k_sbn, 1)
                    emit_mask(p_h1, 0, nc.vector)
                    emit_mask(p_h1, 1, nc.gpsimd)
                    if m + 1 < m4:
                        v4n = emit_proj_v(m + 1, x4n)
                    emit_scale(m, 0, o_h0)
                    o_h1 = emit_AV(m, 1, w_h1, p_h1, v_aug)
                    if m + 1 < m4:
                        v_augn = emit_va(m + 1, v4n)
                    emit_scale(m, 1, o_h1)
                    if m + 1 < m4:
                        x4, qk_sb, k_sb, v_aug = x4n, qk_sbn, k_sbn, v_augn
                    if m % 2 == 1 and m < 7:
                        # drain finished quarter of out_acc via Pool SWDGE
                        # (keeps the HWDGE queue free for input loads)
                        qs = 8 * (m // 2)
                        nc.gpsimd.dma_start(
                            out[:, qs:qs + 8], out_acc[:, qs:qs + 8]
                        )
                    elif m == 6:
                        nc.gpsimd.dma_start(
                            out[:, 24:28], out_acc[:, 24:28]
                        )
                    elif m == 7:
                        # final batches leave in two small DMAs: the second
                        # is issued after the very last ocopy, keep it tiny
                        nc.gpsimd.dma_start(out[:, 28:30], out_acc[:, 28:30])
                        nc.sync.dma_start(out[:, 30:32], out_acc[:, 30:32])

    nc.compile()
    return nc


def prep_in_maps(x, Wk, Wq, Wv):
    """Host-side shard + layout prep. Returns per-core input dicts."""
    import ml_dtypes

    bf16 = ml_dtypes.bfloat16
    x = np.asarray(x, dtype=np.float32)
    scale = np.float32(H) ** np.float32(-0.5)
    wq2 = (np.asarray(Wq, dtype=np.float32) * scale).reshape(3, P, H)
    wk2 = np.asarray(Wk, dtype=np.float32).reshape(3, P, H)
    wqk = np.ascontiguousarray(
        np.concatenate([wq2, wk2], axis=2).transpose(1, 0, 2)
    ).astype(bf16)  # [128, 3, 128]
    wv = np.ascontiguousarray(
        np.asarray(Wv, dtype=np.float32).reshape(3, P, H).transpose(1, 0, 2)
    ).astype(bf16)  # [128, 3, 64]

    in_maps = []
    for c in range(N_CORES):
        xc = x[c * BPC:(c + 1) * BPC]  # [32, 256, 384]
        xt = np.ascontiguousarray(
            xc.reshape(M4, 4, T, 3, P).transpose(0, 4, 1, 3, 2)
        ).astype(bf16)  # [8, 128, 4, 3, 256]
        in_maps.append({"xt": xt, "wqk": wqk, "wv": wv})
    return in_maps


def gather_out(results):
    """Per-core out [128, 32, 2, 65] bf16 (o | rowsum) -> [256, 256, 64] f32."""
    outs = []
    for r in results:
        o = np.asarray(r["out"]).astype(np.float32)  # [128, 32, 2, 65]
        o = o.transpose(1, 2, 0, 3).reshape(BPC, T, H + 1)
        outs.append(o[:, :, 0:H] / o[:, :, H:H + 1])
    return np.concatenate(outs, axis=0).astype(np.float32)


def kernel(x, Wk, Wq, Wv):
    global LAST_RESULT
    from concourse.bass_utils import run_bass_kernel_spmd

    in_maps = prep_in_maps(x, Wk, Wq, Wv)
    nc = _build_nc()
    trace = bool(int(os.environ.get("KERNEL_TRACE", "0")))
    if not trace:
        # The axon NTFF trace path needs antenv.axon_hooks, which this
        # container lacks; make sure an inherited BASS_TRACE can't pull us
        # into it.
        os.environ.setdefault("BASS_NEVER_TRACE", "1")
    res = run_bass_kernel_spmd(
        nc, in_maps, core_ids=list(range(N_CORES)), trace=trace
    )
    LAST_RESULT = res
    return gather_out(res.results)


# revision 47
# speedup vs baseline: 1.2536x; 1.2536x over previous
"""Causal single-head attention (B=256, T=256, C=384, H=64) on 8 trn2 cores.

Data-parallel over batch: each core computes 32 batches independently.

Design (all-bf16 storage, fp32 PSUM accumulate):
  - Host stages x as bf16 in a DMA-friendly layout [8, 128, 4, 3, 256]
    (8 macro-groups of 4 batches; each partition row is 6KB contiguous),
    so input loads are 8 big DMAs instead of 96 small ones. The first
    macro is loaded per-batch so the pipeline fills early.
  - Q and K are projected in ONE packed matmul per (batch, c-chunk):
    stationary Wqk [128c, (q64|k64)] -> PSUM [q rows 0:64 | k rows 64:128].
    One full-width DVE copy drains PSUM to SBUF bf16; a cheap all-SBUF
    partition-shifted copy then brings the k half down to base 0.
  - V is projected in natural [t, h] layout (stationary xT chunk, moving Wv)
    -> no V transpose. The SBUF copy targets v_aug[..., 0:64] whose ones
    column (v_aug[..., 64]=1, prefilled outside the loop) makes the AV
    matmul's last output column the softmax denominator.
  - Scores are computed TRANSPOSED: weiT[s,t] = k q^T (stationary kT chunk,
    moving qT), so exp(weiT) written by ACT directly yields pT in SBUF --
    no P transposes. Causal masking is done multiplicatively AFTER exp
    (logits are bounded so exp never overflows), split between DVE (s0
    diag, gates AV's t0 matmul) and Pool (s1 diag, latency hidden); it
    also fixes the rowsums (masked entries contribute 0).
  - Causal block structure: s0 chunk is needed by all t (free dim 256);
    s1 chunk only by t1 (free 128). The (s1, t0) quadrant of weiT2 is never
    computed or exp'd -- its PSUM space is reused as the AV output tile
    (o lives at weiT2[:, b2, 1, 0:130]), saving a PSUM bank and giving the
    AV output double-buffering for free.
  - The AV output leaves UNNORMALIZED together with its rowsum column
    (65 cols); the softmax division happens on the host after the gather.
    Output accumulates in SBUF bf16 and leaves in overlapped Pool-SWDGE
    DMAs; host transposes back, divides, casts to f32.
  - The main loop is software-pipelined with a one-macro skew: PE order
    per macro is weiT_h0, weiT_h1, proj_qk', AV_h0, proj_qk'', proj_v',
    AV_h1 so exp/mask latency hides under next-macro projection work.
    PSUM accumulation pairs are kept strictly consecutive (interleaving a
    complete start/stop group inside a pending pair corrupts it on HW).

Engine budget per 4-batch macro (cost model): PE ~2885ns, DVE ~2880ns,
ACT ~2590ns, Pool/DMA lower -- PE-bound at ~23us busy, ~41us critical
path (fill + drain + sem latency) per core program.
"""

import os
import sys

import numpy as np

for _p in ("/opt/trn_rl_repo",):
    if _p not in sys.path:
        sys.path.insert(0, _p)

B, T, C, H = 256, 256, 384, 64
N_CORES = 8
BPC = B // N_CORES  # batches per core (32)
P = 128
M4 = BPC // 4  # 4-batch macro groups per core (8)

LAST_RESULT = None  # BassKernelResults of the most recent run (for test.py)


def _build_nc(bpc=BPC, repeats=1):
    import concourse.bacc as bacc
    import concourse.mybir as mybir
    import concourse.tile as tile

    f32 = mybir.dt.float32
    bf16 = mybir.dt.bfloat16
    m4 = bpc // 4

    nc = bacc.Bacc("TRN2", target_bir_lowering=False, debug=False)

    xt = nc.dram_tensor("xt", [m4, P, 4, 3, T], bf16, kind="ExternalInput")
    wqk = nc.dram_tensor("wqk", [P, 3, P], bf16, kind="ExternalInput")
    wv = nc.dram_tensor("wv", [P, 3, H], bf16, kind="ExternalInput")
    out = nc.dram_tensor("out", [P, bpc, 2, H + 1], bf16, kind="ExternalOutput")

    Exp = mybir.ActivationFunctionType.Exp
    mult_op = mybir.AluOpType.mult

    with tile.TileContext(nc) as tc:
        with (
            tc.tile_pool(name="consts", bufs=1) as consts,
            tc.tile_pool(name="inp", bufs=4) as inp,
            tc.tile_pool(name="sb", bufs=3) as sb,
            tc.tile_pool(name="pp", bufs=4) as pp,
            tc.tile_pool(name="va_pool", bufs=2) as va_pool,
            tc.tile_pool(name="acc", bufs=1) as acc,
            tc.tile_pool(name="ps_qk", bufs=1, space="PSUM") as ps_qk,
            tc.tile_pool(name="ps_v", bufs=2, space="PSUM") as ps_v,
            tc.tile_pool(name="ps_w", bufs=2, space="PSUM") as ps_w,
        ):
            # 0/1 upper-triangular (incl diag) bf16 mask: keep s <= t.
            mask01 = consts.tile([P, P], bf16)
            nc.gpsimd.memset(mask01, 1.0)
            nc.gpsimd.affine_select(
                out=mask01,
                in_=mask01,
                compare_op=mybir.AluOpType.is_ge,
                fill=0.0,
                base=0,
                pattern=[[1, P]],
                channel_multiplier=-1,
            )

            wqk_sb = consts.tile([P, 3, P], bf16)
            nc.sync.dma_start(wqk_sb, wqk[:])
            wv_sb = consts.tile([P, 3, H], bf16)

            # Trigger the ACT exp-table load during pipeline fill instead of
            # stalling the first real exp.
            warm = consts.tile([P, 1], f32)
            nc.scalar.activation(warm, mask01[:, 0:1], Exp)

            out_acc = acc.tile([P, bpc, 2, H + 1], bf16)

            # Prefill the ones column of both v_aug buffers; in-loop copies
            # only touch [..., 0:64], so the column survives rotation.
            for _ in range(2):
                va_init = va_pool.tile([P, 4, 2, H + 1], bf16, tag="va")
                nc.gpsimd.memset(va_init[:, :, :, H:H + 1], 1.0)

            import contextlib

            rep_ctx = (
                tc.For_i(0, repeats, 1, hint_engines=(mybir.EngineType.PE,
                                                      mybir.EngineType.DVE,
                                                      mybir.EngineType.Activation,
                                                      mybir.EngineType.Pool,
                                                      mybir.EngineType.SP))
                if repeats > 1
                else contextlib.nullcontext()
            )
            def emit_x_dma(m, split_dma=False, halves=False):
                x4 = inp.tile([P, 4, 3, T], bf16, tag="x4", name=f"x4_{m}")
                if split_dma:
                    for b in range(4):
                        nc.sync.dma_start(x4[:, b], xt[m, :, b])
                elif halves:
                    nc.sync.dma_start(x4[:, 0:2], xt[m, :, 0:2])
                    nc.sync.dma_start(x4[:, 2:4], xt[m, :, 2:4])
                else:
                    nc.sync.dma_start(x4, xt[m])
                return x4

            def alloc_qk4(m):
                return ps_qk.tile([P, 4, T], f32, tag="qk", name=f"qk4_{m}")

            def xsel(m, x4, b):
                return x4[:, b]

            def emit_proj_qk(m, x4, qk4, bs):
                for b in bs:
                    xs = xsel(m, x4, b)
                    for cc in range(3):
                        nc.tensor.matmul(
                            qk4[:, b, :], wqk_sb[:, cc, :], xs[:, cc, :],
                            start=(cc == 0), stop=(cc == 2),
                        )

            def alloc_qk_sb(m):
                qk_sb = sb.tile([P, 4, T], bf16, tag="qk_sb", name=f"qks_{m}")
                k_sb = sb.tile([H, 4, T], bf16, tag="k", name=f"k_{m}")
                return qk_sb, k_sb

            def emit_qk_copy_half(qk4, qk_sb, k_sb, h):
                sl = slice(2 * h, 2 * h + 2)
                nc.vector.tensor_copy(qk_sb[:, sl, :], qk4[:, sl, :])
                nc.vector.tensor_copy(k_sb[:, sl, :], qk_sb[H:P, sl, :])

            def emit_proj_v(m, x4, bs=range(4), v4=None):
                if v4 is None:
                    v4 = ps_v.tile([P, 4, 2, H], f32, tag="v4",
                                   name=f"v4_{m}")
                for b in bs:
                    xs = xsel(m, x4, b)
                    for j in range(2):
                        for cc in range(3):
                            nc.tensor.matmul(
                                v4[:, b, j, :],
                                xs[:, cc, j * P:(j + 1) * P],
                                wv_sb[:, cc, :],
                                start=(cc == 0), stop=(cc == 2),
                            )
                return v4

            def emit_va(m, v4, bs=slice(0, 4), v_aug=None):
                if v_aug is None:
                    v_aug = va_pool.tile([P, 4, 2, H + 1], bf16, tag="va",
                                         name=f"va_{m}")
                nc.scalar.copy(v_aug[:, bs, :, 0:H], v4[:, bs])
                return v_aug

            def emit_weiT(m, half, qk_sb, k_sb):
                weiT2 = ps_w.tile([P, 2, 2, T], f32, tag="w2",
                                  name=f"w2_{m}_{half}")
                # s1 chunks first (tiny, 53ns each): exp_s1 gates the Pool
                # mask -- the longest latency pole -- so start it earliest.
                for b2 in range(2):
                    b = 2 * half + b2
                    nc.tensor.matmul(  # s1 chunk: t1 only (free 128)
                        weiT2[:, b2, 1, P:T],
                        k_sb[:, b, P:T], qk_sb[0:H, b, P:T],
                        start=True, stop=True,
                    )
                for b2 in range(2):
                    b = 2 * half + b2
                    nc.tensor.matmul(  # s0 chunk: all t (free 256)
                        weiT2[:, b2, 0, :],
                        k_sb[:, b, 0:P], qk_sb[0:H, b, :],
                        start=True, stop=True,
                    )
                return weiT2

            def emit_exp(m, half, weiT2):
                pT2 = pp.tile([P, 2, 2, T], bf16, tag="p2",
                              name=f"p2_{m}_{half}")
                # s1-diag first: it gates the Pool mask (the longer pole)
                nc.scalar.activation(
                    pT2[:, :, 1, P:T], weiT2[:, :, 1, P:T], Exp
                )
                nc.scalar.activation(pT2[:, :, 0, :], weiT2[:, :, 0, :], Exp)
                return pT2

            def emit_mask(pT2, j, engine):
                blk = pT2[:, :, j, j * P:(j + 1) * P]
                engine.tensor_tensor(
                    blk, blk,
                    mask01[:, None, :].to_broadcast((P, 2, P)),
                    mult_op,
                )

            def emit_AV(m, half, weiT2, pT2, v_aug):
                # o (AV output incl rowsum col) is carved out of the
                # never-computed (s1, t0) quadrant of weiT2.
                o2 = weiT2[:, :, 1, 0:2 * (H + 1)].rearrange(
                    "p b (j c) -> p b j c", j=2
                )  # [128, 2, 2, 65]
                for b2 in range(2):
                    b = 2 * half + b2
                    # PSUM accumulation pairs MUST be consecutive: a complete
                    # start/stop group interleaved inside a pending pair
                    # corrupts it (verified on HW). Order [t1s0, t1s1, t0]:
                    # pair consecutive AND t0 (latest dep, mask j0) last.
                    nc.tensor.matmul(
                        o2[:, b2, 1, :], pT2[:, b2, 0, P:T],
                        v_aug[:, b, 0, :], start=True, stop=False,
                    )
                    nc.tensor.matmul(
                        o2[:, b2, 1, :], pT2[:, b2, 1, P:T],
                        v_aug[:, b, 1, :], start=False, stop=True,
                    )
                    nc.tensor.matmul(
                        o2[:, b2, 0, :], pT2[:, b2, 0, 0:P],
                        v_aug[:, b, 0, :], start=True, stop=True,
                    )
                return o2

            def emit_scale(m, half, o2):
                # Unnormalized AV output + rowsum column; the softmax
                # division happens on the host after the gather. h0's copy is
                # split DVE/ACT to shave the saturated DVE queue.
                bb = 4 * m + 2 * half
                if half == 0:
                    nc.vector.tensor_copy(out_acc[:, bb:bb + 1, :, :],
                                          o2[:, 0:1])
                    nc.scalar.copy(out_acc[:, bb + 1:bb + 2, :, :], o2[:, 1:2])
                else:
                    nc.vector.tensor_copy(out_acc[:, bb:bb + 2, :, :], o2)

            with rep_ctx:
                # ---- prologue: HALF of macro 0 (batches 0-1 only) so the
                # first weiT/exp chain starts before batches 2-3 even land.
                x4 = emit_x_dma(0, split_dma=True)
                nc.sync.dma_start(wv_sb, wv[:])  # needed later than wqk/x0
                qk4 = alloc_qk4(0)
                qk_sb, k_sb = alloc_qk_sb(0)
                for b in range(2):
                    emit_proj_qk(0, x4, qk4, (b,))
                    nc.vector.tensor_copy(qk_sb[:, b:b + 1, :],
                                          qk4[:, b:b + 1, :])
                    nc.vector.tensor_copy(k_sb[:, b:b + 1, :],
                                          qk_sb[H:P, b:b + 1, :])
                v4 = emit_proj_v(0, x4, bs=(0, 1))
                v_aug = emit_va(0, v4, bs=slice(0, 2))
                x4n_pre = emit_x_dma(1, halves=True)  # prefetch macro 1
                v_rest_pending = True

                # ---- steady state: W/A of m interleaved with P of m+1 ----
                # PE order/macro: weiT_h0, weiT_h1, proj_qk', AV_h0,
                # proj_v', AV_h1 -- dependent pairs separated by filler so
                # exp/mask latency is hidden.  DVE order: mask0_h0, qk_copy',
                # k_shift', mask0_h1, recip/scale x2.
                for m in range(m4):
                    w_h0 = emit_weiT(m, 0, qk_sb, k_sb)
                    p_h0 = emit_exp(m, 0, w_h0)
                    emit_mask(p_h0, 0, nc.vector)
                    emit_mask(p_h0, 1, nc.gpsimd)
                    if m == 0 and v_rest_pending:
                        # macro-0 second half (qk+v for batches 2-3) as
                        # filler under the first exp/mask latency
                        for b in (2, 3):
                            emit_proj_qk(0, x4, qk4, (b,))
                            nc.vector.tensor_copy(qk_sb[:, b:b + 1, :],
                                                  qk4[:, b:b + 1, :])
                            nc.vector.tensor_copy(k_sb[:, b:b + 1, :],
                                                  qk_sb[H:P, b:b + 1, :])
                        emit_proj_v(0, x4, bs=(2, 3), v4=v4)
                        emit_va(0, v4, bs=slice(2, 4), v_aug=v_aug)
                        v_rest_pending = False
                    w_h1 = emit_weiT(m, 1, qk_sb, k_sb)
                    p_h1 = emit_exp(m, 1, w_h1)
                    if m + 1 < m4:
                        x4n = x4n_pre
                        if m + 2 < m4:
                            x4n_pre = emit_x_dma(m + 2, halves=True)
                        qk4n = alloc_qk4(m + 1)
                        emit_proj_qk(m + 1, x4n, qk4n, (0, 1))
                        qk_sbn, k_sbn = alloc_qk_sb(m + 1)
                        emit_qk_copy_half(qk4n, qk_sbn, k_sbn, 0)
                    o_h0 = emit_AV(m, 0, w_h0, p_h0, v_aug)
                    if m + 1 < m4:
                        emit_proj_qk(m + 1, x4n, qk4n, (2, 3))
                        emit_qk_copy_half(qk4n, qk_sbn, k_sbn, 1)
                    emit_mask(p_h1, 0, nc.vector)
                    emit_mask(p_h1, 1, nc.gpsimd)
                    if m + 1 < m4:
                        v4n = emit_proj_v(m + 1, x4n)
                    emit_scale(m, 0, o_h0)
                    o_h1 = emit_AV(m, 1, w_h1, p_h1, v_aug)
                    if m + 1 < m4:
                        v_augn = emit_va(m + 1, v4n)
                    emit_scale(m, 1, o_h1)
                    if m + 1 < m4:
                        x4, qk_sb, k_sb, v_aug = x4n, qk_sbn, k_sbn, v_augn
                    if m % 2 == 1 and m < 7:
                        # drain finished quarter of out_acc via Pool SWDGE
                        # (keeps the HWDGE queue free for input loads)
                        qs = 8 * (m // 2)
                        nc.gpsimd.dma_start(
                            out[:, qs:qs + 8], out_acc[:, qs:qs + 8]
                        )
                    elif m == 6:
                        nc.gpsimd.dma_start(
                            out[:, 24:28], out_acc[:, 24:28]
                        )
                    elif m == 7:
                        # final batches leave in two small DMAs: the second
                        # is issued after the very last ocopy, keep it tiny
                        nc.gpsimd.dma_start(out[:, 28:30], out_acc[:, 28:30])
                        nc.sync.dma_start(out[:, 30:32], out_acc[:, 30:32])

    nc.compile()
    return nc


def prep_in_maps(x, Wk, Wq, Wv):
    """Host-side shard + layout prep. Returns per-core input dicts."""
    import ml_dtypes

    bf16 = ml_dtypes.bfloat16
    x = np.asarray(x, dtype=np.float32)
    scale = np.float32(H) ** np.float32(-0.5)
    wq2 = (np.asarray(Wq, dtype=np.float32) * scale).reshape(3, P, H)
    wk2 = np.asarray(Wk, dtype=np.float32).reshape(3, P, H)
    wqk = np.ascontiguousarray(
        np.concatenate([wq2, wk2], axis=2).transpose(1, 0, 2)
    ).astype(bf16)  # [128, 3, 128]
    wv = np.ascontiguousarray(
        np.asarray(Wv, dtype=np.float32).reshape(3, P, H).transpose(1, 0, 2)
    ).astype(bf16)  # [128, 3, 64]

    in_maps = []
    for c in range(N_CORES):
        xc = x[c * BPC:(c + 1) * BPC]  # [32, 256, 384]
        xt = np.ascontiguousarray(
            xc.reshape(M4, 4, T, 3, P).transpose(0, 4, 1, 3, 2)
        ).astype(bf16)  # [8, 128, 4, 3, 256]
        in_maps.append({"xt": xt, "wqk": wqk, "wv": wv})
    return in_maps


def gather_out(results):
    """Per-core out [128, 32, 2, 65] bf16 (o | rowsum) -> [256, 256, 64] f32."""
    outs = []
    for r in results:
        o = np.asarray(r["out"]).astype(np.float32)  # [128, 32, 2, 65]
        o = o.transpose(1, 2, 0, 3).reshape(BPC, T, H + 1)
        outs.append(o[:, :, 0:H] / o[:, :, H:H + 1])
    return np.concatenate(outs, axis=0).astype(np.float32)


def kernel(x, Wk, Wq, Wv):
    global LAST_RESULT
    from concourse.bass_utils import run_bass_kernel_spmd

    in_maps = prep_in_maps(x, Wk, Wq, Wv)
    nc = _build_nc()
    trace = bool(int(os.environ.get("KERNEL_TRACE", "0")))
    if not trace:
        # The axon NTFF trace path needs antenv.axon_hooks, which this
        # container lacks; make sure an inherited BASS_TRACE can't pull us
        # into it.
        os.environ.setdefault("BASS_NEVER_TRACE", "1")
    res = run_bass_kernel_spmd(
        nc, in_maps, core_ids=list(range(N_CORES)), trace=trace
    )
    LAST_RESULT = res
    return gather_out(res.results)
